# revision 1
# baseline (speedup 1.0000x reference)
"""Trainium2 Bass kernel for nn_Eq2to2_58815282152312 (PELICAN Eq2to2 layer).

Math (per batch n, x_cf[d,i,j] = inputs[n,i,j,d], c_b = coefs[:,:,b]):
  out[i,j,s] = lrelu( sum_d c3[d,s] x_cf[d,i,j] + sum_d c4[d,s] x_cf[d,j,i]
                      + A[i,s] + CC[j,s] + delta_ij * E[i,s] ) * mask
A/CC/E are small [128,64] maps from diag/rowsum/colsum/trace/allsum (the 13
cheap basis ops), computed on host. The delta term (diagonal pixels) is
patched on host (128 pixels/batch).

Device: everything is PE matmuls into PSUM, K=128, base partition 0:
  XI[128, 16384] bf16: rows 0-63 = x^T[d, pixel], rows 64-127 = x^T[d, swapped]
  per 512-pixel chunk c (pixel = i*128+j, i = 4c + col//128, j = col%128):
    MM1: [c3;c4]^T @ XI_chunk            (Y1 + Y2, bf16)
    MM2: A-rows^T @ IND4  (accumulate)   (+A[i,s], fp32, K=4)
    MM3: CC^T    @ INDJ  (accumulate)    (+CC[j,s], fp32)
  ACT: LeakyReLU PSUM -> SBUF (paired into [128,512] tiles), DMA out.
Output outT[s, pixel]; host transposes back. 1 batch per core, 8 cores.
"""
import sys
import numpy as np

sys.path.insert(0, "/opt/trn_rl_repo")

import ml_dtypes

B, N, C, BASIS = 8, 128, 64, 15
AVG = 49.0
SLOPE = 0.01

_cache = {}


def _build_bass():
    from concourse import bass, mybir

    f32 = mybir.dt.float32
    bf16 = mybir.dt.bfloat16
    NCH = 32  # 512-pixel chunks
    NB = 4  # psum-slot / out-tile ping-pong depth

    nc = bass.Bass()
    xi_d = nc.dram_tensor("xi", [128, 16384], bf16, kind="ExternalInput")
    w_d = nc.dram_tensor("w", [128, 64], bf16, kind="ExternalInput")
    wa_d = nc.dram_tensor("wa", [4, 2048], f32, kind="ExternalInput")
    cm_d = nc.dram_tensor("cm", [128, 64], f32, kind="ExternalInput")
    ind4_d = nc.dram_tensor("ind4", [4, 512], f32, kind="ExternalInput")
    indj_d = nc.dram_tensor("indj", [128, 512], f32, kind="ExternalInput")
    out_d = nc.dram_tensor("outT", [64, 32, 512], f32, kind="ExternalOutput")

    with (
        nc.sbuf_tensor([128, 16384], bf16) as xi,
        nc.sbuf_tensor([128, 64], bf16) as wt,
        nc.sbuf_tensor([4, 2048], f32) as wa,
        nc.sbuf_tensor([128, 64], f32) as cm,
        nc.sbuf_tensor([4, 512], f32) as ind4,
        nc.sbuf_tensor([128, 512], f32) as indj,
        nc.sbuf_tensor([128, NB * 512], f32) as ot,
        nc.psum_tensor([64, NB * 512], f32) as ps,
        nc.semaphore("s_in") as s_in,
        nc.semaphore("s_mm") as s_mm,
        nc.semaphore("s_act") as s_act,
        nc.semaphore("s_out0") as s_out0,
        nc.semaphore("s_out1") as s_out1,
        nc.semaphore("s_out2") as s_out2,
        nc.semaphore("s_out3") as s_out3,
        nc.Block() as block,
    ):
        s_out = [s_out0, s_out1, s_out2, s_out3]

        @block.sync
        def _(sync):
            sync.dma_start(out=wt[:], in_=w_d[:]).then_inc(s_in, 16)
            sync.dma_start(out=wa[:], in_=wa_d[:]).then_inc(s_in, 16)
            sync.dma_start(out=cm[:], in_=cm_d[:]).then_inc(s_in, 16)
            sync.dma_start(out=ind4[:], in_=ind4_d[:]).then_inc(s_in, 16)
            sync.dma_start(out=indj[:], in_=indj_d[:]).then_inc(s_in, 16)
            sync.dma_start(out=xi[:], in_=xi_d[:]).then_inc(s_in, 16)
            for p in range(16):
                b = p % NB
                sync.wait_ge(s_act, 2 * p + 2)
                dst = out_d[:, 2 * p : 2 * p + 2, :].rearrange("s c f -> c s f")
                sync.dma_start(out=dst, in_=ot[:, b * 512 : (b + 1) * 512]).then_inc(
                    s_out[b], 16
                )

        @block.tensor
        def _(tensor):
            tensor.wait_ge(s_in, 96)
            for c in range(NCH):
                b = c % NB
                if c >= NB:
                    # psum slot free once ACT(c-NB) has read it
                    tensor.wait_ge(s_act, c - NB + 1)
                pt = ps[:, b * 512 : (b + 1) * 512]
                tensor.matmul(
                    pt,
                    wt[:, :],
                    xi[:, c * 512 : (c + 1) * 512],
                    start=True,
                    stop=False,
                )
                tensor.matmul(
                    pt,
                    wa[:, c * 64 : (c + 1) * 64],
                    ind4[:, :],
                    start=False,
                    stop=False,
                )
                tensor.matmul(
                    pt, cm[:, :], indj[:, :], start=False, stop=True
                ).then_inc(s_mm, 1)

        @block.scalar
        def _(scalar):
            for c in range(NCH):
                b = c % NB
                p = c // 2
                scalar.wait_ge(s_mm, c + 1)
                if p >= NB and c % 2 == 0:
                    # out tile slot free once its pair-(p-NB) store completed
                    scalar.wait_ge(s_out[p % NB], 16 * (p // NB))
                po = 64 * (c % 2)
                otb = ot[po : po + 64, (p % NB) * 512 : (p % NB + 1) * 512]
                scalar.activation(
                    otb,
                    ps[:, b * 512 : (b + 1) * 512],
                    mybir.ActivationFunctionType.Lrelu,
                    alpha=SLOPE,
                ).then_inc(s_act, 1)

    return nc


def _get_nc():
    if "nc" not in _cache:
        _cache["nc"] = _build_bass()
    return _cache["nc"]


def _prep(inputs_arr, coefs00, coefs01, coefs10, coefs11, bias, diag_bias):
    """Host prep: per-batch aux tensors + device input maps."""
    coefs = (
        coefs00[:, None, :] * coefs10[:, :, None]
        + coefs01[None, :, :] * coefs11[:, :, None]
    )  # [d, s, 15]
    c = [np.ascontiguousarray(coefs[:, :, b]) for b in range(BASIS)]

    x_cf = np.ascontiguousarray(inputs_arr.transpose(0, 3, 1, 2))  # [B,d,i,j]
    diag = np.ascontiguousarray(np.diagonal(x_cf, axis1=2, axis2=3))  # [B,d,i]
    rowsum = x_cf.sum(3) / AVG
    colsum = x_cf.sum(2) / AVG
    trace = diag.sum(2) / AVG
    allsum = x_cf.sum((2, 3)) / (AVG * AVG)

    def proj(stat, cb):  # [B,d,i] x [d,s] -> [B,i,s]
        return np.einsum("ndi,ds->nis", stat, cb, optimize=True)

    K0 = trace @ c[13] + allsum @ c[14]  # [B, s]
    A = (
        proj(diag, c[1]) + proj(rowsum, c[9]) + proj(colsum, c[11])
        + K0[:, None, :] + bias[None, None, :]
    )  # [B, i, s]
    CC = proj(diag, c[2]) + proj(rowsum, c[10]) + proj(colsum, c[12])  # [B,j,s]
    K1 = trace @ c[7] + allsum @ c[8]
    E = (
        proj(diag, c[0]) + proj(rowsum, c[5]) + proj(colsum, c[6])
        + K1[:, None, :] + diag_bias[None, None, :]
    )  # [B, i, s]

    # host diagonal values (pre-activation)
    zdiag = proj(diag, c[3] + c[4]) + A + CC + E  # [B, i, s]
    outdiag = np.where(zdiag >= 0, zdiag, SLOPE * zdiag).astype(np.float32)

    wmat = np.concatenate([c[3], c[4]], 0).astype(ml_dtypes.bfloat16)  # [128,64]
    col = np.arange(512)
    ind4 = (col[None, :] // 128 == np.arange(4)[:, None]).astype(np.float32)
    indj = np.tile(np.eye(128, dtype=np.float32), (1, 4))

    in_maps = []
    for n in range(B):
        xi = np.empty((128, 16384), ml_dtypes.bfloat16)
        xi[0:64] = x_cf[n].reshape(64, 16384)
        xi[64:128] = x_cf[n].transpose(0, 2, 1).reshape(64, 16384)
        wa = (
            A[n].reshape(32, 4, 64).transpose(1, 0, 2).reshape(4, 2048)
        ).astype(np.float32)
        in_maps.append(
            {
                "xi": xi,
                "w": wmat,
                "wa": np.ascontiguousarray(wa),
                "cm": CC[n].astype(np.float32),
                "ind4": ind4,
                "indj": indj,
            }
        )
    return in_maps, outdiag


def _gather(results, outdiag, mask):
    out = np.empty((B, N, N, C), np.float32)
    idx = np.arange(N)
    for n in range(B):
        ot = np.asarray(results[n]["outT"], np.float32).reshape(64, 16384)
        out[n] = ot.reshape(64, 128, 128).transpose(1, 2, 0)
        out[n][idx, idx, :] = outdiag[n]
    return out * mask


def run_device(in_maps, trace=False):
    from concourse.bass_utils import run_bass_kernel_spmd

    nc = _get_nc()
    return run_bass_kernel_spmd(nc, in_maps, list(range(B)), trace=trace)


def kernel(
    inputs, mask, nobj, coefs00, coefs01, coefs10, coefs11, bias, diag_bias
):
    inputs = np.asarray(inputs, np.float32)
    mask = np.asarray(mask, np.float32)
    in_maps, outdiag = _prep(
        inputs,
        np.asarray(coefs00, np.float32),
        np.asarray(coefs01, np.float32),
        np.asarray(coefs10, np.float32),
        np.asarray(coefs11, np.float32),
        np.asarray(bias, np.float32),
        np.asarray(diag_bias, np.float32),
    )
    res = run_device(in_maps, trace=False)
    return _gather(res.results, outdiag, mask)



# revision 5
# speedup vs baseline: 3172.7332x; 3172.7332x over previous
"""Trainium2 Bass kernel for nn_Eq2to2_58815282152312 (PELICAN Eq2to2 layer).

Math (per batch n, x_cf[d,i,j] = inputs[n,i,j,d], c_b = coefs[:,:,b]):
  out[i,j,s] = lrelu( sum_d c3[d,s] x[d,i,j] + sum_d c4[d,s] x[d,j,i]
                      + A[i,s] + CC[j,s] + delta_ij * E[i,s] ) * mask
A/CC/E are small [128,64] maps from diag/rowsum/colsum/trace/allsum (the 13
cheap basis ops), computed on host. The delta term (diagonal pixels) is
patched on host (128 pixels/batch).

Device (per core, 1 batch, 32 chunks of 512 pixels; pixel = i*128+j,
chunk c covers i in [4c, 4c+4), col = (i-4c)*128 + j):
  MM_a: K=68 bf16  [c3 ; A-rows]^T @ [x ; i-indicator]      (start)
  MM_b: K=64 bf16  c4^T @ x-swap-view  (reads x[d,j,i] via strided AP)
  CC add: for CC_ON_PE chunks a third K=128 bf16 matmul cm^T @ indj;
          for the rest a DVE tensor_tensor add of CC^T (f32) into PSUM.
  ACT:  one Lrelu per 4-chunk quad, PSUM -> SBUF bf16; DMA out per quad.
Output out[s, pixel] bf16; host transposes back, patches the diagonal with
f32 host values, applies mask. 1 batch per core, 8 cores.
"""
import sys
import numpy as np

sys.path.insert(0, "/opt/trn_rl_repo")

import ml_dtypes

B, N, C, BASIS = 8, 128, 64, 15
AVG = 49.0
SLOPE = 0.01

NCH = 32          # 512-pixel chunks
IB = 4            # i-rows per chunk
KA = C + IB       # MM_a contraction depth
NB = 8            # psum slots (1 bank each)
QUAD = 4          # chunks per activation/out-DMA
NQ = NCH // QUAD  # 8 quads
NOUT = 2          # out-tile slots (1 quad each)
# chunks whose CC-add runs on PE as a third matmul (rest on DVE)
CC_ON_PE = frozenset(c for c in range(NCH) if c % 3 == 2)

_cache = {}


def _ready_targets():
    """Per chunk: (which_sem, target) marking PSUM ready for activation."""
    ready = []
    ndve = 0
    for c in range(NCH):
        if c in CC_ON_PE:
            ready.append(("mm", c + 1))
        else:
            ndve += 1
            ready.append(("cc", ndve))
    return ready


def _build_bass(act_copy=False, reps=1):
    """act_copy=True swaps Lrelu -> Copy so CoreSim's value executor (which
    lacks Lrelu) can run the kernel for race/numeric checks.
    reps>1 repeats the whole body (re-DMAing inputs, serialized between
    reps) so a wall-clock slope over reps isolates per-exec device time."""
    from concourse import bass, mybir

    f32 = mybir.dt.float32
    bf16 = mybir.dt.bfloat16
    act_fn = (
        mybir.ActivationFunctionType.Copy
        if act_copy
        else mybir.ActivationFunctionType.Lrelu
    )

    nc = bass.Bass()
    xi_d = nc.dram_tensor("xi", [KA, 16384], bf16, kind="ExternalInput")
    wb_d = nc.dram_tensor("wb", [KA, NCH * 64], bf16, kind="ExternalInput")
    w2_d = nc.dram_tensor("w2", [C, 64], bf16, kind="ExternalInput")
    cc_d = nc.dram_tensor("cc", [64, 512], f32, kind="ExternalInput")
    cm_d = nc.dram_tensor("cm", [128, 64], bf16, kind="ExternalInput")
    ij_d = nc.dram_tensor("ij", [128, 512], bf16, kind="ExternalInput")
    out_d = nc.dram_tensor("outT", [64, 16384], bf16, kind="ExternalOutput")

    ready = _ready_targets()

    with (
        nc.sbuf_tensor([KA, 16384], bf16) as xi,
        nc.sbuf_tensor([KA, NCH * 64], bf16) as wb,
        nc.sbuf_tensor([C, 64], bf16) as w2,
        nc.sbuf_tensor([64, 512], f32) as cc,
        nc.sbuf_tensor([128, 64], bf16) as cm,
        nc.sbuf_tensor([128, 512], bf16) as ij,
        nc.sbuf_tensor([64, NOUT * QUAD * 512], bf16) as ot,
        nc.psum_tensor([64, NB * 512], f32) as ps,
        nc.semaphore("s_in") as s_in,
        nc.semaphore("s_mm") as s_mm,
        nc.semaphore("s_cc") as s_cc,
        nc.semaphore("s_act") as s_act,
        nc.semaphore("s_out0") as s_out0,
        nc.semaphore("s_out1") as s_out1,
        nc.Block() as block,
    ):
        s_out = [s_out0, s_out1]

        @block.sync
        def _(sync):
            for r in range(reps):
                if r > 0:
                    sync.wait_ge(s_act, NQ * r)
                for t_d, t_s in [
                    (xi_d, xi), (wb_d, wb), (w2_d, w2),
                    (cc_d, cc), (cm_d, cm), (ij_d, ij),
                ]:
                    sync.dma_start(out=t_s[:], in_=t_d[:]).then_inc(s_in, 16)
                for q in range(NQ):
                    qg = NQ * r + q
                    o = qg % NOUT
                    sync.wait_ge(s_act, qg + 1)
                    sync.dma_start(
                        out=out_d[:, q * 2048 : (q + 1) * 2048],
                        in_=ot[:, o * 2048 : (o + 1) * 2048],
                    ).then_inc(s_out[o], 16)

        @block.tensor
        def _(tensor):
            xsw = xi[0:C, :].rearrange("p (j i) -> p i j", j=128, i=128)
            for r in range(reps):
                tensor.wait_ge(s_in, 96 * (r + 1))
                for c in range(NCH):
                    g = NCH * r + c
                    b = c % NB
                    if g >= NB:
                        # slot reuse: quad holding chunk g-NB must be activated
                        tensor.wait_ge(s_act, g // QUAD - 1)
                    pt = ps[:, b * 512 : (b + 1) * 512]
                    tensor.matmul(
                        pt,
                        wb[:, c * 64 : (c + 1) * 64],
                        xi[:, c * 512 : (c + 1) * 512],
                        start=True,
                        stop=False,
                    )
                    on_pe = c in CC_ON_PE
                    mm = tensor.matmul(
                        pt,
                        w2[:, :],
                        xsw[:, c * IB : (c + 1) * IB, :],
                        start=False,
                        stop=not on_pe,
                    )
                    if on_pe:
                        mm = tensor.matmul(
                            pt, cm[:, :], ij[:, :], start=False, stop=True
                        )
                    mm.then_inc(s_mm, 1)

        @block.vector
        def _(vector):
            for r in range(reps):
                for c in range(NCH):
                    if c in CC_ON_PE:
                        continue
                    g = NCH * r + c
                    b = c % NB
                    pt = ps[:, b * 512 : (b + 1) * 512]
                    vector.wait_ge(s_mm, g + 1)
                    vector.tensor_tensor(
                        pt, pt, cc[:, :], mybir.AluOpType.add
                    ).then_inc(s_cc, 1)

        @block.scalar
        def _(scalar):
            ndve_rep = NCH - len(CC_ON_PE)
            for r in range(reps):
                for q in range(NQ):
                    qg = NQ * r + q
                    for c in range(q * QUAD, (q + 1) * QUAD):
                        sem, tgt = ready[c]
                        off = NCH * r if sem == "mm" else ndve_rep * r
                        scalar.wait_ge(s_mm if sem == "mm" else s_cc, tgt + off)
                    o = qg % NOUT
                    if qg >= NOUT:
                        # out-tile slot reuse: DMA of quad qg-NOUT must be done
                        scalar.wait_ge(s_out[o], 16 * (qg // NOUT))
                    pb = (q % (NB // QUAD)) * QUAD * 512
                    kwargs = {} if act_copy else {"alpha": SLOPE}
                    scalar.activation(
                        ot[:, o * 2048 : (o + 1) * 2048],
                        ps[:, pb : pb + 2048],
                        act_fn,
                        **kwargs,
                    ).then_inc(s_act, 1)

    return nc


def _get_nc(reps=1):
    key = ("nc", reps)
    if key not in _cache:
        _cache[key] = _build_bass(reps=reps)
    return _cache[key]


def _prep(inputs_arr, coefs00, coefs01, coefs10, coefs11, bias, diag_bias):
    """Host prep: per-batch aux tensors + device input maps."""
    coefs = (
        coefs00[:, None, :] * coefs10[:, :, None]
        + coefs01[None, :, :] * coefs11[:, :, None]
    )  # [d, s, 15]
    c = [np.ascontiguousarray(coefs[:, :, b]) for b in range(BASIS)]

    x_cf = np.ascontiguousarray(inputs_arr.transpose(0, 3, 1, 2))  # [B,d,i,j]
    diag = np.ascontiguousarray(np.diagonal(x_cf, axis1=2, axis2=3))  # [B,d,i]
    rowsum = x_cf.sum(3) / AVG
    colsum = x_cf.sum(2) / AVG
    trace = diag.sum(2) / AVG
    allsum = x_cf.sum((2, 3)) / (AVG * AVG)

    def proj(stat, cb):  # [B,d,i] x [d,s] -> [B,i,s]
        return np.einsum("ndi,ds->nis", stat, cb, optimize=True)

    K0 = trace @ c[13] + allsum @ c[14]  # [B, s]
    A = (
        proj(diag, c[1]) + proj(rowsum, c[9]) + proj(colsum, c[11])
        + K0[:, None, :] + bias[None, None, :]
    )  # [B, i, s]
    CC = proj(diag, c[2]) + proj(rowsum, c[10]) + proj(colsum, c[12])  # [B,j,s]
    K1 = trace @ c[7] + allsum @ c[8]
    E = (
        proj(diag, c[0]) + proj(rowsum, c[5]) + proj(colsum, c[6])
        + K1[:, None, :] + diag_bias[None, None, :]
    )  # [B, i, s]

    # host diagonal values (pre-activation)
    zdiag = proj(diag, c[3] + c[4]) + A + CC + E  # [B, i, s]
    outdiag = np.where(zdiag >= 0, zdiag, SLOPE * zdiag).astype(np.float32)

    bf = ml_dtypes.bfloat16
    col = np.arange(16384)
    indmod = np.stack(
        [((col // 128) % IB == k).astype(np.float32) for k in range(IB)]
    )  # [4, 16384]
    ij = np.tile(np.eye(128, dtype=np.float32), (1, IB)).astype(bf)
    w2 = c[4].astype(bf)

    in_maps = []
    for n in range(B):
        xi = np.empty((KA, 16384), bf)
        xi[0:C] = x_cf[n].reshape(C, 16384)
        xi[C:KA] = indmod
        wb = np.empty((KA, NCH * 64), bf)
        wb[0:C] = np.tile(c[3], (1, NCH))
        wb[C:KA] = (
            A[n].reshape(NCH, IB, 64).transpose(1, 0, 2).reshape(IB, NCH * 64)
        )
        cct = np.ascontiguousarray(CC[n].T)  # [s, j]
        in_maps.append(
            {
                "xi": xi,
                "wb": wb,
                "w2": w2,
                "cc": np.ascontiguousarray(np.tile(cct, (1, IB)), np.float32),
                "cm": CC[n].astype(bf),
                "ij": ij,
            }
        )
    return in_maps, outdiag


def _gather(results, outdiag, mask):
    out = np.empty((B, N, N, C), np.float32)
    idx = np.arange(N)
    for n in range(B):
        ot = np.asarray(results[n]["outT"]).astype(np.float32)
        out[n] = ot.reshape(C, N, N).transpose(1, 2, 0)
        out[n][idx, idx, :] = outdiag[n]
    return out * mask


def run_device(in_maps, trace=False):
    from concourse.bass_utils import run_bass_kernel_spmd

    nc = _get_nc()
    return run_bass_kernel_spmd(nc, in_maps, list(range(B)), trace=trace)


def kernel(
    inputs, mask, nobj, coefs00, coefs01, coefs10, coefs11, bias, diag_bias
):
    inputs = np.asarray(inputs, np.float32)
    mask = np.asarray(mask, np.float32)
    in_maps, outdiag = _prep(
        inputs,
        np.asarray(coefs00, np.float32),
        np.asarray(coefs01, np.float32),
        np.asarray(coefs10, np.float32),
        np.asarray(coefs11, np.float32),
        np.asarray(bias, np.float32),
        np.asarray(diag_bias, np.float32),
    )
    res = run_device(in_maps, trace=False)
    return _gather(res.results, outdiag, mask)
